# revision 17
# baseline (speedup 1.0000x reference)
"""Trainium2 Bass kernel for the 5x5-neighborhood min-L1 loss (nn_NNLoss).

Computation (faithful to the reference):
    gt_pad = pad(ground_truth, rows by nw//2, cols by nh//2, value=-10000)
    norms[b,h,w,s] = sum_c |gt_pad[b,c,h+di,w+dj] - predicted[b,c,h,w]|
                     for s=(di,dj), di in range(nh), dj in range(nw)
    loss = mean over (b,h,w) of min_s norms

Sharding: pure data parallel over the batch dim: 16 images -> 2 per core
across 8 NeuronCores.  Each core returns per-partition partial sums
[128,1]; the host adds them up and divides (the scalar "all-reduce").

Per-core layout (fp16 compute):
  - partition dim = 128 H-rows (2 row-blocks cover H=256)
  - free dim fuses (channel, image, W): chunk q = c*IPC + img
  - row shifts (di) are materialized as `nh` row-shifted copies of the
    padded ground truth (DMA cannot be replaced by AP partition offsets:
    DVE lanes are per-partition), column shifts (dj) are free-dim AP
    offsets; two column parities are kept so the 16-bit 2x DVE mode
    (needs 4B-aligned starts) works for odd dj.
  - per shift: sub (DVE) -> abs (ACT) -> channel-sum (DVE) -> running
    min (DVE).  Final free-dim reduce -> [128,1] fp32 partials.
"""

import os

# The execution path needs the axon PJRT platform; a harness that pins
# JAX_PLATFORMS=cpu would hide the NeuronCores from jax.
if "axon" not in os.environ.get("JAX_PLATFORMS", "axon"):
    os.environ.pop("JAX_PLATFORMS", None)

import numpy as np

B, C, H, W = 16, 3, 256, 256
N_CORES = 8
IPC = B // N_CORES  # images per core
PAD_VAL = -10000.0

_BUILD_CACHE = {}
LAST_EXEC_NS = [None]  # exec_time_ns of the last traced run (for test.py)


def _build(nh, nw):
    """Trace the Bass/Tile program for one core. Returns the Bass object."""
    from contextlib import ExitStack

    import concourse.bacc as bacc
    import concourse.bass as bass  # noqa: F401
    import concourse.tile as tile
    from concourse import mybir
    from concourse.alu_op_type import AluOpType

    f32 = mybir.dt.float32
    f16 = mybir.dt.float16
    Abs = mybir.ActivationFunctionType.Abs
    Copy = mybir.ActivationFunctionType.Copy

    W_PAD = nh // 2  # pads the W (column) dim -- faithful swap vs torch
    H_PAD = nw // 2  # pads the H (row) dim
    NDI, NDJ = nh, nw  # row / column shift counts
    WP = W + 2 * W_PAD  # padded row width (260)
    Q = C * IPC  # fused (channel, image) chunks: 6
    FD = Q * W  # 1536
    FDP = Q * WP  # 1560
    SW = IPC * W  # 512: per-channel chunk width in the fused free dim
    assert H % 128 == 0
    NBLK = H // 128

    # Bacc (not raw Bass): its compile() splits multi-wait instructions
    # (TRN2 allows at most one sync wait per instruction) among other
    # required lowerings.
    nc = bacc.Bacc("TRN2", target_bir_lowering=False, debug=False)
    pred_d = nc.dram_tensor("predicted", [IPC, C, H, W], f32, kind="ExternalInput")
    gt_d = nc.dram_tensor("ground_truth", [IPC, C, H, W], f32, kind="ExternalInput")
    out_d = nc.dram_tensor("partials", [128, 1], f32, kind="ExternalOutput")

    with tile.TileContext(nc) as tc, ExitStack() as ctx:
        p_stage = ctx.enter_context(tc.tile_pool(name="p_stage", bufs=2))
        p_pool = ctx.enter_context(tc.tile_pool(name="pred", bufs=1))
        g_stage = ctx.enter_context(tc.tile_pool(name="g_stage", bufs=4))
        g_pool = ctx.enter_context(tc.tile_pool(name="gsel", bufs=1))
        d_pool = ctx.enter_context(tc.tile_pool(name="d", bufs=3))
        a_pool = ctx.enter_context(tc.tile_pool(name="a", bufs=3))
        s_pool = ctx.enter_context(tc.tile_pool(name="s", bufs=2))
        m_pool = ctx.enter_context(tc.tile_pool(name="m", bufs=1))
        r_pool = ctx.enter_context(tc.tile_pool(name="r", bufs=1))

        r_tiles = []
        for b in range(NBLK):
            h0 = 128 * b

            # ---- predicted: load fp32, convert to fp16 ----
            # chunks are img-major (q = img*C + ch) so the whole block is
            # ONE DMA: the (img, ch) axes merge to a uniform-stride dim on
            # the DRAM side.  One DMA -> one sync wait on the consumer
            # (the codegen rejects instructions with too many waits).
            ps = p_stage.tile([128, FD], f32, tag="p_stage")
            nc.sync.dma_start(
                ps.rearrange("p (q w) -> p q w", q=Q),
                pred_d.ap().rearrange("i c h w -> h (i c) w")[h0 : h0 + 128],
            )
            pt = p_pool.tile([128, FD], f16, tag=f"pred{b}")
            nc.scalar.activation(pt[:, :], ps[:, :], Copy)
            ptv = pt.rearrange("p (q w) -> p q w", q=Q)

            # ---- ground truth: nh row-shifted fp16 copies, 2 col parities ----
            gsel = {}
            for di in range(NDI):
                # tile row p holds gt_pad row (h0 + p + di)
                p0 = max(0, H_PAD - h0 - di)
                p1 = min(127, H - 1 + H_PAD - h0 - di)
                r0 = h0 + p0 + di - H_PAD
                cnt = p1 - p0 + 1

                gs = g_stage.tile([128, FDP], f32, tag="g_stage")
                gsv = gs.rearrange("p (q w) -> p q w", q=Q)
                # pad columns (left/right of each chunk) and pad rows are
                # memset on the fp32 staging tile BEFORE the DMAs (compute
                # engines need quadrant-aligned partition starts, so pad
                # rows use 32-row strips that the DMAs then overwrite);
                # the converts propagate the pads into both parity tiles.
                nc.gpsimd.memset(gsv[:, :, 0:W_PAD], PAD_VAL)
                nc.gpsimd.memset(gsv[:, :, W_PAD + W : WP], PAD_VAL)
                if p0 > 0:
                    nc.gpsimd.memset(gs[0:32, :], PAD_VAL)
                if p1 < 127:
                    nc.gpsimd.memset(gs[96:128, :], PAD_VAL)
                nc.sync.dma_start(
                    gsv[p0 : p1 + 1, :, W_PAD : W_PAD + W],
                    gt_d.ap().rearrange("i c h w -> h (i c) w")[r0 : r0 + cnt],
                )
                for par in range(2):
                    g = g_pool.tile([128, FDP], f16, tag=f"g{b}_{di}_{par}")
                    gv = g.rearrange("p (q w) -> p q w", q=Q)
                    if par == 0:
                        nc.gpsimd.tensor_copy(g[:, :], gs[:, :])
                    else:
                        # parity-1 tile: col w holds gt_pad col (w+1)
                        nc.gpsimd.tensor_copy(
                            gv[:, :, 0 : WP - 1], gsv[:, :, 1:WP]
                        )
                    gsel[(di, par)] = gv

            # ---- the nh*nw shift loop ----
            m = m_pool.tile([128, SW], f16, tag=f"m{b}")
            first = True
            for di in range(NDI):
                for dj in range(NDJ):
                    par = dj & 1
                    off = dj - par
                    gv = gsel[(di, par)]
                    d = d_pool.tile([128, FD], f16, tag="d")
                    dv = d.rearrange("p (q w) -> p q w", q=Q)
                    nc.vector.tensor_sub(dv, gv[:, :, off : off + W], ptv)
                    a = a_pool.tile([128, FD], f16, tag="a")
                    nc.scalar.activation(a[:, :], d[:, :], Abs)
                    # channel sum: chunks are img-major, so channel slices
                    # are strided views [p, IPC, W]
                    a4 = a.rearrange("p (i c w) -> p i c w", i=IPC, c=C)
                    s01 = s_pool.tile([128, SW], f16, tag="s01")
                    s01v = s01.rearrange("p (i w) -> p i w", i=IPC)
                    nc.vector.tensor_add(s01v, a4[:, :, 0, :], a4[:, :, 1, :])
                    if first:
                        mv = m.rearrange("p (i w) -> p i w", i=IPC)
                        nc.vector.tensor_add(mv, s01v, a4[:, :, 2, :])
                        first = False
                    else:
                        sf = s_pool.tile([128, SW], f16, tag="sf")
                        sfv = sf.rearrange("p (i w) -> p i w", i=IPC)
                        nc.vector.tensor_add(sfv, s01v, a4[:, :, 2, :])
                        nc.vector.tensor_tensor(m, m, sf, AluOpType.min)

            r = r_pool.tile([128, 1], f32, tag=f"r{b}")
            nc.vector.tensor_reduce(r, m, mybir.AxisListType.X, AluOpType.add)
            r_tiles.append(r)

        tot = r_tiles[0]
        for b in range(1, NBLK):
            nxt = r_pool.tile([128, 1], f32, tag=f"tot{b}")
            nc.vector.tensor_add(nxt, tot, r_tiles[b])
            tot = nxt
        nc.sync.dma_start(out_d.ap()[:, :], tot)

    nc.compile()
    return nc


def _get_nc(nh, nw):
    key = (nh, nw)
    if key not in _BUILD_CACHE:
        _BUILD_CACHE[key] = _build(nh, nw)
    return _BUILD_CACHE[key]


def _setup_trace():
    """Register the axon NTFF profile hook (the image's antenv lacks
    axon_hooks) and stub the artifact upload so trace=True works."""
    import sys
    import types

    from concourse import bass_utils

    try:
        import antenv.axon_hooks  # noqa: F401
    except ImportError:
        try:
            import trn_agent_boot.trn_boot as tb

            hook = tb._ntff_profile_via_ctypes("/opt/axon/libaxon_pjrt.so")
            mod = types.ModuleType("antenv.axon_hooks")
            mod.get_axon_ntff_profile_hook = lambda: hook
            sys.modules["antenv.axon_hooks"] = mod
        except Exception as e:  # profiling is best-effort
            print(f"ntff hook setup failed: {e}")
            return False
    bass_utils.upload_artifacts = lambda tmpdir: f"local:{tmpdir}"
    return True


def kernel(predicted, ground_truth, nh=5, nw=5):
    from concourse import bass_utils

    nh, nw = int(nh), int(nw)
    pred = np.ascontiguousarray(np.asarray(predicted, dtype=np.float32))
    gt = np.ascontiguousarray(np.asarray(ground_truth, dtype=np.float32))
    assert pred.shape == (B, C, H, W) and gt.shape == (B, C, H, W)

    nc = _get_nc(nh, nw)
    in_maps = [
        {
            "predicted": pred[k * IPC : (k + 1) * IPC],
            "ground_truth": gt[k * IPC : (k + 1) * IPC],
        }
        for k in range(N_CORES)
    ]
    trace = bool(int(os.environ.get("NNLOSS_TRACE", "0")))
    if trace:
        trace = _setup_trace()
    res = bass_utils.run_bass_kernel_spmd(
        nc, in_maps, list(range(N_CORES)), trace=trace
    )
    LAST_EXEC_NS[0] = res.exec_time_ns
    total = 0.0
    for r in res.results:
        total += float(np.asarray(r["partials"], dtype=np.float64).sum())
    return np.float32(total / (B * H * W))


# revision 19
# speedup vs baseline: 1.2011x; 1.2011x over previous
"""Trainium2 Bass kernel for the 5x5-neighborhood min-L1 loss (nn_NNLoss).

Computation (faithful to the reference):
    gt_pad = pad(ground_truth, rows by nw//2, cols by nh//2, value=-10000)
    norms[b,h,w,s] = sum_c |gt_pad[b,c,h+di,w+dj] - predicted[b,c,h,w]|
                     for s=(di,dj), di in range(nh), dj in range(nw)
    loss = mean over (b,h,w) of min_s norms

Sharding: pure data parallel over the batch dim: 16 images -> 2 per core
across 8 NeuronCores.  Each core returns per-partition partial sums
[128,1]; the host adds them up and divides (the scalar "all-reduce").

Per-core layout (fp16 compute):
  - partition dim = 128 H-rows (2 row-blocks cover H=256)
  - free dim fuses (channel, image, W): chunk q = c*IPC + img
  - row shifts (di) are materialized as `nh` row-shifted copies of the
    padded ground truth (DMA cannot be replaced by AP partition offsets:
    DVE lanes are per-partition), column shifts (dj) are free-dim AP
    offsets; two column parities are kept so the 16-bit 2x DVE mode
    (needs 4B-aligned starts) works for odd dj.
  - per shift: sub (DVE) -> abs (ACT) -> channel-sum (DVE) -> running
    min (DVE).  Final free-dim reduce -> [128,1] fp32 partials.
"""

import os

# The execution path needs the axon PJRT platform; a harness that pins
# JAX_PLATFORMS=cpu would hide the NeuronCores from jax.
if "axon" not in os.environ.get("JAX_PLATFORMS", "axon"):
    os.environ.pop("JAX_PLATFORMS", None)

import numpy as np

B, C, H, W = 16, 3, 256, 256
N_CORES = 8
IPC = B // N_CORES  # images per core
PAD_VAL = -10000.0

_BUILD_CACHE = {}
LAST_EXEC_NS = [None]  # exec_time_ns of the last traced run (for test.py)


def _build(nh, nw):
    """Trace the Bass/Tile program for one core. Returns the Bass object."""
    from contextlib import ExitStack

    import concourse.bacc as bacc
    import concourse.bass as bass  # noqa: F401
    import concourse.tile as tile
    from concourse import mybir
    from concourse.alu_op_type import AluOpType

    f32 = mybir.dt.float32
    # bf16, not fp16: the DVE's 2x tensor_tensor packing mode only has
    # uops for bf16 (fp16 measured at 1x on HW)
    f16 = mybir.dt.bfloat16
    Abs = mybir.ActivationFunctionType.Abs
    Copy = mybir.ActivationFunctionType.Copy

    W_PAD = nh // 2  # pads the W (column) dim -- faithful swap vs torch
    H_PAD = nw // 2  # pads the H (row) dim
    NDI, NDJ = nh, nw  # row / column shift counts
    WP = W + 2 * W_PAD  # padded row width (260)
    Q = C * IPC  # fused (channel, image) chunks: 6
    FD = Q * W  # 1536
    FDP = Q * WP  # 1560
    SW = IPC * W  # 512: per-channel chunk width in the fused free dim
    assert H % 128 == 0
    NBLK = H // 128

    # Bacc (not raw Bass): its compile() splits multi-wait instructions
    # (TRN2 allows at most one sync wait per instruction) among other
    # required lowerings.
    nc = bacc.Bacc("TRN2", target_bir_lowering=False, debug=False)
    pred_d = nc.dram_tensor("predicted", [IPC, C, H, W], f32, kind="ExternalInput")
    gt_d = nc.dram_tensor("ground_truth", [IPC, C, H, W], f32, kind="ExternalInput")
    out_d = nc.dram_tensor("partials", [128, 1], f32, kind="ExternalOutput")

    with tile.TileContext(nc) as tc, ExitStack() as ctx:
        p_stage = ctx.enter_context(tc.tile_pool(name="p_stage", bufs=2))
        p_pool = ctx.enter_context(tc.tile_pool(name="pred", bufs=1))
        g_stage = ctx.enter_context(tc.tile_pool(name="g_stage", bufs=4))
        g_pool = ctx.enter_context(tc.tile_pool(name="gsel", bufs=1))
        d_pool = ctx.enter_context(tc.tile_pool(name="d", bufs=3))
        a_pool = ctx.enter_context(tc.tile_pool(name="a", bufs=3))
        s_pool = ctx.enter_context(tc.tile_pool(name="s", bufs=2))
        m_pool = ctx.enter_context(tc.tile_pool(name="m", bufs=1))
        r_pool = ctx.enter_context(tc.tile_pool(name="r", bufs=1))

        r_tiles = []
        for b in range(NBLK):
            h0 = 128 * b

            # ---- predicted: load fp32, convert to fp16 ----
            # chunks are img-major (q = img*C + ch) so the whole block is
            # ONE DMA: the (img, ch) axes merge to a uniform-stride dim on
            # the DRAM side.  One DMA -> one sync wait on the consumer
            # (the codegen rejects instructions with too many waits).
            ps = p_stage.tile([128, FD], f32, tag="p_stage")
            nc.sync.dma_start(
                ps.rearrange("p (q w) -> p q w", q=Q),
                pred_d.ap().rearrange("i c h w -> h (i c) w")[h0 : h0 + 128],
            )
            pt = p_pool.tile([128, FD], f16, tag=f"pred{b}")
            nc.scalar.activation(pt[:, :], ps[:, :], Copy)
            ptv = pt.rearrange("p (q w) -> p q w", q=Q)

            # ---- ground truth: nh row-shifted fp16 copies, 2 col parities ----
            gsel = {}
            for di in range(NDI):
                # tile row p holds gt_pad row (h0 + p + di)
                p0 = max(0, H_PAD - h0 - di)
                p1 = min(127, H - 1 + H_PAD - h0 - di)
                r0 = h0 + p0 + di - H_PAD
                cnt = p1 - p0 + 1

                gs = g_stage.tile([128, FDP], f32, tag="g_stage")
                gsv = gs.rearrange("p (q w) -> p q w", q=Q)
                # pad columns (left/right of each chunk) and pad rows are
                # memset on the fp32 staging tile BEFORE the DMAs (compute
                # engines need quadrant-aligned partition starts, so pad
                # rows use 32-row strips that the DMAs then overwrite);
                # the converts propagate the pads into both parity tiles.
                nc.gpsimd.memset(gsv[:, :, 0:W_PAD], PAD_VAL)
                nc.gpsimd.memset(gsv[:, :, W_PAD + W : WP], PAD_VAL)
                if p0 > 0:
                    nc.gpsimd.memset(gs[0:32, :], PAD_VAL)
                if p1 < 127:
                    nc.gpsimd.memset(gs[96:128, :], PAD_VAL)
                nc.sync.dma_start(
                    gsv[p0 : p1 + 1, :, W_PAD : W_PAD + W],
                    gt_d.ap().rearrange("i c h w -> h (i c) w")[r0 : r0 + cnt],
                )
                # par0: fp32->bf16 cast on ACT (gpsimd CAST measured 8us!)
                g0 = g_pool.tile([128, FDP], f16, tag=f"g{b}_{di}_0")
                g0v = g0.rearrange("p (q w) -> p q w", q=Q)
                nc.scalar.activation(g0[:, :], gs[:, :], Copy)
                gsel[(di, 0)] = g0v
                # par1 (col w holds gt_pad col w+1): same-dtype shifted
                # copy on gpsimd from the bf16 par0 tile
                g1 = g_pool.tile([128, FDP], f16, tag=f"g{b}_{di}_1")
                g1v = g1.rearrange("p (q w) -> p q w", q=Q)
                nc.gpsimd.tensor_copy(g1v[:, :, 0 : WP - 1], g0v[:, :, 1:WP])
                gsel[(di, 1)] = g1v

            # ---- the nh*nw shift loop ----
            m = m_pool.tile([128, SW], f16, tag=f"m{b}")
            first = True
            for di in range(NDI):
                for dj in range(NDJ):
                    par = dj & 1
                    off = dj - par
                    gv = gsel[(di, par)]
                    d = d_pool.tile([128, FD], f16, tag="d")
                    dv = d.rearrange("p (q w) -> p q w", q=Q)
                    nc.vector.tensor_sub(dv, gv[:, :, off : off + W], ptv)
                    a = a_pool.tile([128, FD], f16, tag="a")
                    nc.scalar.activation(a[:, :], d[:, :], Abs)
                    # channel sum: chunks are img-major, so channel slices
                    # are strided views [p, IPC, W]
                    a4 = a.rearrange("p (i c w) -> p i c w", i=IPC, c=C)
                    s01 = s_pool.tile([128, SW], f16, tag="s01")
                    s01v = s01.rearrange("p (i w) -> p i w", i=IPC)
                    nc.vector.tensor_add(s01v, a4[:, :, 0, :], a4[:, :, 1, :])
                    if first:
                        mv = m.rearrange("p (i w) -> p i w", i=IPC)
                        nc.vector.tensor_add(mv, s01v, a4[:, :, 2, :])
                        first = False
                    else:
                        sf = s_pool.tile([128, SW], f16, tag="sf")
                        sfv = sf.rearrange("p (i w) -> p i w", i=IPC)
                        nc.vector.tensor_add(sfv, s01v, a4[:, :, 2, :])
                        nc.vector.tensor_tensor(m, m, sf, AluOpType.min)

            r = r_pool.tile([128, 1], f32, tag=f"r{b}")
            nc.vector.tensor_reduce(r, m, mybir.AxisListType.X, AluOpType.add)
            r_tiles.append(r)

        tot = r_tiles[0]
        for b in range(1, NBLK):
            nxt = r_pool.tile([128, 1], f32, tag=f"tot{b}")
            nc.vector.tensor_add(nxt, tot, r_tiles[b])
            tot = nxt
        nc.sync.dma_start(out_d.ap()[:, :], tot)

    nc.compile()
    return nc


def _get_nc(nh, nw):
    key = (nh, nw)
    if key not in _BUILD_CACHE:
        _BUILD_CACHE[key] = _build(nh, nw)
    return _BUILD_CACHE[key]


def _setup_trace():
    """Register the axon NTFF profile hook (the image's antenv lacks
    axon_hooks) and stub the artifact upload so trace=True works."""
    import sys
    import types

    from concourse import bass_utils

    try:
        import antenv.axon_hooks  # noqa: F401
    except ImportError:
        try:
            import trn_agent_boot.trn_boot as tb

            hook = tb._ntff_profile_via_ctypes("/opt/axon/libaxon_pjrt.so")
            mod = types.ModuleType("antenv.axon_hooks")
            mod.get_axon_ntff_profile_hook = lambda: hook
            sys.modules["antenv.axon_hooks"] = mod
        except Exception as e:  # profiling is best-effort
            print(f"ntff hook setup failed: {e}")
            return False
    bass_utils.upload_artifacts = lambda tmpdir: f"local:{tmpdir}"
    return True


def kernel(predicted, ground_truth, nh=5, nw=5):
    from concourse import bass_utils

    nh, nw = int(nh), int(nw)
    pred = np.ascontiguousarray(np.asarray(predicted, dtype=np.float32))
    gt = np.ascontiguousarray(np.asarray(ground_truth, dtype=np.float32))
    assert pred.shape == (B, C, H, W) and gt.shape == (B, C, H, W)

    nc = _get_nc(nh, nw)
    in_maps = [
        {
            "predicted": pred[k * IPC : (k + 1) * IPC],
            "ground_truth": gt[k * IPC : (k + 1) * IPC],
        }
        for k in range(N_CORES)
    ]
    trace = bool(int(os.environ.get("NNLOSS_TRACE", "0")))
    if trace:
        trace = _setup_trace()
    res = bass_utils.run_bass_kernel_spmd(
        nc, in_maps, list(range(N_CORES)), trace=trace
    )
    LAST_EXEC_NS[0] = res.exec_time_ns
    total = 0.0
    for r in res.results:
        total += float(np.asarray(r["partials"], dtype=np.float64).sum())
    return np.float32(total / (B * H * W))


# revision 22
# speedup vs baseline: 1.3744x; 1.1443x over previous
"""Trainium2 Bass kernel for the 5x5-neighborhood min-L1 loss (nn_NNLoss).

Computation (faithful to the reference):
    gt_pad = pad(ground_truth, rows by nw//2, cols by nh//2, value=-10000)
    norms[b,h,w,s] = sum_c |gt_pad[b,c,h+di,w+dj] - predicted[b,c,h,w]|
                     for s=(di,dj), di in range(nh), dj in range(nw)
    loss = mean over (b,h,w) of min_s norms

Sharding: pure data parallel over the batch dim: 16 images -> 2 per core
across 8 NeuronCores.  Each core returns per-partition partial sums
[128,1]; the host adds them up and divides (the scalar "all-reduce").

Per-core layout (fp16 compute):
  - partition dim = 128 H-rows (2 row-blocks cover H=256)
  - free dim fuses (channel, image, W): chunk q = c*IPC + img
  - row shifts (di) are materialized as `nh` row-shifted copies of the
    padded ground truth (DMA cannot be replaced by AP partition offsets:
    DVE lanes are per-partition), column shifts (dj) are free-dim AP
    offsets; two column parities are kept so the 16-bit 2x DVE mode
    (needs 4B-aligned starts) works for odd dj.
  - per shift: sub (DVE) -> abs (ACT) -> channel-sum (DVE) -> running
    min (DVE).  Final free-dim reduce -> [128,1] fp32 partials.
"""

import os

# The execution path needs the axon PJRT platform; a harness that pins
# JAX_PLATFORMS=cpu would hide the NeuronCores from jax.
if "axon" not in os.environ.get("JAX_PLATFORMS", "axon"):
    os.environ.pop("JAX_PLATFORMS", None)

import numpy as np

B, C, H, W = 16, 3, 256, 256
N_CORES = 8
IPC = B // N_CORES  # images per core
PAD_VAL = -10000.0

_BUILD_CACHE = {}
LAST_EXEC_NS = [None]  # exec_time_ns of the last traced run (for test.py)


def _build(nh, nw):
    """Trace the Bass/Tile program for one core. Returns the Bass object."""
    from contextlib import ExitStack

    import concourse.bacc as bacc
    import concourse.bass as bass  # noqa: F401
    import concourse.tile as tile
    from concourse import mybir
    from concourse.alu_op_type import AluOpType

    f32 = mybir.dt.float32
    # bf16, not fp16: the DVE's 2x tensor_tensor packing mode only has
    # uops for bf16 (fp16 measured at 1x on HW)
    f16 = mybir.dt.bfloat16
    Abs = mybir.ActivationFunctionType.Abs
    Copy = mybir.ActivationFunctionType.Copy

    W_PAD = nh // 2  # pads the W (column) dim -- faithful swap vs torch
    H_PAD = nw // 2  # pads the H (row) dim
    NDI, NDJ = nh, nw  # row / column shift counts
    WP = W + 2 * W_PAD  # padded row width (260)
    Q = C * IPC  # fused (channel, image) chunks: 6
    FD = Q * W  # 1536
    FDP = Q * WP  # 1560
    SW = IPC * W  # 512: per-channel chunk width in the fused free dim
    assert H % 128 == 0
    NBLK = H // 128

    # Bacc (not raw Bass): its compile() splits multi-wait instructions
    # (TRN2 allows at most one sync wait per instruction) among other
    # required lowerings.
    nc = bacc.Bacc("TRN2", target_bir_lowering=False, debug=False)
    pred_d = nc.dram_tensor("predicted", [IPC, C, H, W], f32, kind="ExternalInput")
    gt_d = nc.dram_tensor("ground_truth", [IPC, C, H, W], f32, kind="ExternalInput")
    out_d = nc.dram_tensor("partials", [128, 1], f32, kind="ExternalOutput")

    import bass_rust as _br

    def strided(ap, levels, extra_offset=0):
        """Hand-built free-dim AP on an existing [128, N] view (keeps the
        partition level and base offset; used for the overlapping
        dj-window axis and the 0-stride pred broadcast)."""
        c = ap.copy()
        c.ap = _br.VecI64Pair([list(ap.ap[0])] + [list(l) for l in levels])
        if extra_offset:
            c.offset = c.offset + extra_offset
        return c

    G = NDJ  # all column shifts merged into one wide instruction group

    with tile.TileContext(nc) as tc, ExitStack() as ctx:
        p_stage = ctx.enter_context(tc.tile_pool(name="p_stage", bufs=2))
        p_pool = ctx.enter_context(tc.tile_pool(name="pred", bufs=1))
        g_stage = ctx.enter_context(tc.tile_pool(name="g_stage", bufs=3))
        g_pool = ctx.enter_context(tc.tile_pool(name="gsel", bufs=1))
        d_pool = ctx.enter_context(tc.tile_pool(name="d", bufs=2))
        s_pool = ctx.enter_context(tc.tile_pool(name="s", bufs=2))
        m_pool = ctx.enter_context(tc.tile_pool(name="m", bufs=1))
        r_pool = ctx.enter_context(tc.tile_pool(name="r", bufs=1))

        r_tiles = []
        for b in range(NBLK):
            h0 = 128 * b

            # ---- predicted: one DMA (img-major chunks merge on the DRAM
            # side), fp32 -> bf16 cast on ACT ----
            ps = p_stage.tile([128, FD], f32, tag="p_stage")
            nc.sync.dma_start(
                ps.rearrange("p (q w) -> p q w", q=Q),
                pred_d.ap().rearrange("i c h w -> h (i c) w")[h0 : h0 + 128],
            )
            pt = p_pool.tile([128, FD], f16, tag=f"pred{b}")
            nc.scalar.activation(pt[:, :], ps[:, :], Copy)
            # broadcast view: [p, G(stride 0), Q, W]
            ptb = strided(pt[:, :], [[0, G], [W, Q], [1, W]])

            m = None
            for di in range(NDI):
                # tile row p holds gt_pad row (h0 + p + di)
                p0 = max(0, H_PAD - h0 - di)
                p1 = min(127, H - 1 + H_PAD - h0 - di)
                r0 = h0 + p0 + di - H_PAD
                cnt = p1 - p0 + 1

                gs = g_stage.tile([128, FDP], f32, tag="g_stage")
                gsv = gs.rearrange("p (q w) -> p q w", q=Q)
                # pad columns / pad rows memset BEFORE the DMA (quadrant-
                # aligned partition strips; DMA overwrites the interior);
                # the cast propagates pads into the bf16 tile.
                nc.gpsimd.memset(gsv[:, :, 0:W_PAD], PAD_VAL)
                nc.gpsimd.memset(gsv[:, :, W_PAD + W : WP], PAD_VAL)
                if p0 > 0:
                    nc.gpsimd.memset(gs[0:32, :], PAD_VAL)
                if p1 < 127:
                    nc.gpsimd.memset(gs[96:128, :], PAD_VAL)
                nc.sync.dma_start(
                    gsv[p0 : p1 + 1, :, W_PAD : W_PAD + W],
                    gt_d.ap().rearrange("i c h w -> h (i c) w")[r0 : r0 + cnt],
                )
                g0 = g_pool.tile([128, FDP], f16, tag=f"g{b}_{di}")
                nc.scalar.activation(g0[:, :], gs[:, :], Copy)

                # ---- all NDJ column shifts in ONE instruction group ----
                # gt operand: overlapping window axis [1, G] (odd offsets
                # measured penalty-free on HW)
                gt_op = strided(g0[:, :], [[1, G], [WP, Q], [1, W]])
                dG = d_pool.tile([128, G * FD], f16, tag="d")
                d_out = strided(dG[:, :], [[FD, G], [W, Q], [1, W]])
                nc.vector.tensor_sub(d_out, gt_op, ptb)
                # |d| in place on ACT (1x but off the DVE critical path)
                nc.scalar.activation(dG[:, :], dG[:, :], Abs)
                # channel sum: chunks are img-major (q = i*C + c), so the
                # c-slices are [G, IPC, W] strided views at offset c*W
                CW = C * W
                dc = [
                    strided(dG[:, :], [[FD, G], [CW, IPC], [1, W]], c * W)
                    for c in range(C)
                ]
                s01 = s_pool.tile([128, G * SW], f16, tag="s01")
                s01v = strided(s01[:, :], [[SW, G], [W, IPC], [1, W]])
                nc.vector.tensor_add(s01v, dc[0], dc[1])
                sG = s_pool.tile([128, G * SW], f16, tag="sG")
                sGv = strided(sG[:, :], [[SW, G], [W, IPC], [1, W]])
                nc.vector.tensor_add(sGv, s01v, dc[2])
                # running min, [128, SW] slices (wide MIN measured slow)
                sl = [sG[:, g * SW : (g + 1) * SW] for g in range(G)]
                k = 0
                if m is None:
                    m = m_pool.tile([128, SW], f16, tag=f"m{b}")
                    if G >= 2:
                        nc.vector.tensor_tensor(m, sl[0], sl[1], AluOpType.min)
                        k = 2
                    else:
                        nc.vector.tensor_copy(m, sl[0])
                        k = 1
                for g in range(k, G):
                    nc.vector.tensor_tensor(m, m, sl[g], AluOpType.min)

            r = r_pool.tile([128, 1], f32, tag=f"r{b}")
            nc.vector.tensor_reduce(r, m, mybir.AxisListType.X, AluOpType.add)
            r_tiles.append(r)

        tot = r_tiles[0]
        for b in range(1, NBLK):
            nxt = r_pool.tile([128, 1], f32, tag=f"tot{b}")
            nc.vector.tensor_add(nxt, tot, r_tiles[b])
            tot = nxt
        nc.sync.dma_start(out_d.ap()[:, :], tot)

    nc.compile()
    return nc


def _get_nc(nh, nw):
    key = (nh, nw)
    if key not in _BUILD_CACHE:
        _BUILD_CACHE[key] = _build(nh, nw)
    return _BUILD_CACHE[key]


def _setup_trace():
    """Register the axon NTFF profile hook (the image's antenv lacks
    axon_hooks) and stub the artifact upload so trace=True works."""
    import sys
    import types

    from concourse import bass_utils

    try:
        import antenv.axon_hooks  # noqa: F401
    except ImportError:
        try:
            import trn_agent_boot.trn_boot as tb

            hook = tb._ntff_profile_via_ctypes("/opt/axon/libaxon_pjrt.so")
            mod = types.ModuleType("antenv.axon_hooks")
            mod.get_axon_ntff_profile_hook = lambda: hook
            sys.modules["antenv.axon_hooks"] = mod
        except Exception as e:  # profiling is best-effort
            print(f"ntff hook setup failed: {e}")
            return False
    bass_utils.upload_artifacts = lambda tmpdir: f"local:{tmpdir}"
    return True


def kernel(predicted, ground_truth, nh=5, nw=5):
    from concourse import bass_utils

    nh, nw = int(nh), int(nw)
    pred = np.ascontiguousarray(np.asarray(predicted, dtype=np.float32))
    gt = np.ascontiguousarray(np.asarray(ground_truth, dtype=np.float32))
    assert pred.shape == (B, C, H, W) and gt.shape == (B, C, H, W)

    nc = _get_nc(nh, nw)
    in_maps = [
        {
            "predicted": pred[k * IPC : (k + 1) * IPC],
            "ground_truth": gt[k * IPC : (k + 1) * IPC],
        }
        for k in range(N_CORES)
    ]
    trace = bool(int(os.environ.get("NNLOSS_TRACE", "0")))
    if trace:
        trace = _setup_trace()
    res = bass_utils.run_bass_kernel_spmd(
        nc, in_maps, list(range(N_CORES)), trace=trace
    )
    LAST_EXEC_NS[0] = res.exec_time_ns
    total = 0.0
    for r in res.results:
        total += float(np.asarray(r["partials"], dtype=np.float64).sum())
    return np.float32(total / (B * H * W))
